# revision 10
# baseline (speedup 1.0000x reference)
"""MoE grouped-GEMM expert FFN (SwiGLU) for Trainium2, 8-core expert parallelism.

Contract: kernel(**inputs) takes FULL unsharded inputs, returns FULL output.

Strategy:
  - Host-side routing: tokens are contiguous per expert; split expert groups
    into chunks, band-assign chunks across 8 cores with an identical
    segment-capacity structure on every core (SPMD: one Bass program).
  - Per core, per segment: local GEMM1 (x @ w1w3) -> SwiGLU -> GEMM2 (h @ w2).
  - Host-side combine: scatter per-core output rows back to full output.

Matmul dtype is configurable (MM_DT): float32r runs at full PE rate with
~2.5e-4 rel err; float16 additionally halves DMA bytes and enables fast
weight load, at ~1e-3 rel err. PSUM/silu stay fp32; the output is stored
fp16 (quantization ~2.4e-4 of absmax) and cast back to fp32 on host.

Layout choices:
  - All device inputs are host-repacked so every DMA loads long contiguous
    rows with few instructions (DMA issue costs ~0.6-1.3us per instruction
    on the sync sequencer; per-engine DMA bandwidth scales with run length).
  - x: packed per token tile as [tile, 128, 8*512] (hidden chunk k on the
    free dim) -> 1 DMA per token tile.
  - w1w3: columns permuted so psum chunk c holds gate[64c:64c+64] on
    partitions 0:64 and up on 64:128 (SwiGLU = partition-slice op); rows
    packed as [S, 4, 128, 2*1408] (k-chunk pairs) -> 4 DMAs per segment.
  - w2: rows packed as [S, 128, 6*1024] (j on free dim; j=5 has 64 valid
    rows) -> 1 DMA per segment.
  - GEMM1 iterates k (contraction) outer / m inner within m-groups of <=4 so
    compute starts after ~1MB of DMA and segment boundaries pipeline.
  - GEMM2 uses h as stationary ([inter, token] slices) and w2 as moving ->
    output lands token-major in PSUM and stores contiguously (via gpsimd
    queue to keep the sync sequencer free for loads).
  - GEMM2 of segment s is deferred and interleaved between GEMM1 m-groups
    of segment s+1: small segments' GEMM1 is LDWEIGHTS-bound (88 loads of
    ~88ns vs <90ns of streaming each), and the interleaved GEMM2 matmuls
    (218ns streams) keep the PE busy while loads are pulled ahead into the
    background weight buffer.
"""

import numpy as np

import concourse.bacc as bacc
import concourse.mybir as mybir
from concourse import tile
from concourse.bass_utils import run_bass_kernel_spmd

HIDDEN = 1024
INTER = 704
N_EXPERTS = 32
NCORES = 8
KC = HIDDEN // 128  # 8 k-chunks over hidden
MC = (2 * INTER) // 128  # 11 m-chunks over permuted gate|up dim
JC = (INTER + 127) // 128  # 6 j-chunks over inter for GEMM2 (last is 64 rows)
TT = 512  # token tile (moving free dim)
M_GROUPS = [(0, 2), (2, 4), (4, 6), (6, 8), (8, 10), (10, 11)]  # pair-sized m-groups

f32 = mybir.dt.float32
f16 = mybir.dt.float16

# Matmul input dtype. float16 runs ~1.7x faster than float32r at ~4.6e-4
# rel err (vs 2.5e-4 for f32r); PSUM accumulation is fp32 either way.
MM_DT = mybir.dt.float16
NP_DT = np.float16
ESZ = 2  # element size of MM_DT in bytes
OUT_F16 = True  # store output fp16 (halves store DMA), cast to fp32 on host


def set_dtype(name):
    global MM_DT, NP_DT, ESZ
    if name == "f32r":
        MM_DT, NP_DT, ESZ = mybir.dt.float32r, np.float32, 4
    elif name == "f16":
        MM_DT, NP_DT, ESZ = mybir.dt.float16, np.float16, 2
    elif name == "bf16":
        MM_DT, NP_DT, ESZ = mybir.dt.bfloat16, np.float32, 2  # cast via jax-free trick
    else:
        raise ValueError(name)


# Column permutation of w1w3's last dim (2*INTER): m-chunks come in
# (gate, up) pairs of full 128-row blocks so SwiGLU runs full-width
# [128, tt] ACT/DVE ops (engine cost scales with free size, not partition
# count). chunk 2j = gate[128j:128j+128], chunk 2j+1 = up[128j:128j+128]
# for j<5; the last chunk holds the 64-row tails [gate[640:704]|up[640:704]].
_PERM = np.empty(2 * INTER, dtype=np.int64)
for _j in range(5):
    _PERM[256 * _j : 256 * _j + 128] = np.arange(128 * _j, 128 * _j + 128)
    _PERM[256 * _j + 128 : 256 * _j + 256] = INTER + np.arange(
        128 * _j, 128 * _j + 128
    )
_PERM[1280:1344] = np.arange(640, 704)
_PERM[1344:1408] = INTER + np.arange(640, 704)


def _to_np_dt(a):
    """Cast fp32 array to the host dtype for MM_DT."""
    if MM_DT == mybir.dt.bfloat16:
        b = np.asarray(a, dtype=np.float32).copy()
        v = b.view(np.uint32)
        v += 0x8000  # round-to-nearest-even-ish
        v &= 0xFFFF0000
        return b
    return np.asarray(a, dtype=NP_DT)


def _make_chunks(counts, starts, tmax):
    chunks = []  # (n, expert, tok_start)
    for e in range(N_EXPERTS):
        n = int(counts[e])
        a = int(starts[e])
        if n <= 0:
            continue
        nparts = -(-n // tmax)
        base, rem = divmod(n, nparts)
        off = 0
        for p in range(nparts):
            ln = base + (1 if p < rem else 0)
            if ln > 0:
                chunks.append((ln, e, a + off))
                off += ln
    return chunks


def _pe_time(caps):
    """Predicted PE critical time for a segment-cap structure (fp16)."""
    col = 0.427e-9  # per moving column at 2.4GHz
    ld = 88e-9  # LDWEIGHTS (measured on HW, FWL fp16)
    t = 0.0
    for C in caps:
        for t0 in range(0, C, TT):
            tt = min(TT, C - t0)
            t += 88 * max(tt * col, ld)  # GEMM1: 8k x 11m stationary loads
            t += -(-tt // 128) * 12 * max(512 * col, ld)  # GEMM2 chunks
    return t


def _dma_time(S, cap_total):
    w_seg = (HIDDEN * 2 * INTER + 768 * HIDDEN) * ESZ  # w13 + padded w2
    out_b = 2 if OUT_F16 else 4
    return (S * w_seg + cap_total * HIDDEN * (ESZ + out_b)) / 390e9


def _plan(counts):
    """Balance (expert, token-chunk) pieces across NCORES cores.

    Chunks are sorted by size and dealt in bands of 8 (one per core): slot s
    capacity = the largest chunk in band s, which minimizes total capacity
    for a given chunk multiset. The split threshold trades segment count
    (weight DMA traffic) against padding (PE + activation traffic).
    """
    starts = np.zeros(N_EXPERTS, dtype=np.int64)
    np.cumsum(counts[:-1], out=starts[1:])

    best = None
    for tmax in (4096, 2048, 1536, 1024, *range(256, 1025, 16)):
        chunks = _make_chunks(counts, starts, max(1, tmax))
        if not chunks:
            chunks = [(0, None, 0)]
        chunks.sort(key=lambda c: -c[0])
        S = -(-len(chunks) // NCORES)
        caps = []
        for s in range(S):
            band = chunks[NCORES * s : NCORES * (s + 1)]
            caps.append(max(8, ((band[0][0] + 7) // 8) * 8))
        cap_total = sum(caps)
        dma_t = _dma_time(S, cap_total)
        pe_t = _pe_time(caps)
        score = max(dma_t, pe_t) + 0.2 * min(dma_t, pe_t)
        if best is None or score < best[0]:
            best = (score, chunks, S, caps)

    _, chunks, S, caps = best
    offs = np.concatenate([[0], np.cumsum(caps)[:-1]]).astype(np.int64)
    cap_total = int(sum(caps))

    assign = [[] for _ in range(NCORES)]
    for s in range(S):
        band = chunks[NCORES * s : NCORES * (s + 1)]
        for c in range(NCORES):
            if c < len(band):
                n, e, a = band[c]
                assign[c].append((e, a, n))
            else:
                assign[c].append((None, 0, 0))
    return assign, caps, offs, cap_total


def _tiles_of(caps):
    """Token tiles as (segment, t0, tt) in execution order."""
    out = []
    for s, C in enumerate(caps):
        for t0 in range(0, C, TT):
            out.append((s, t0, min(TT, C - t0)))
    return out


def _build(S, caps, cap_total):
    """Build the SPMD Bass program for one core's segment structure."""
    nc = bacc.Bacc(
        "TRN2",
        target_bir_lowering=False,
        debug=False,
        enable_asserts=False,
        num_devices=NCORES,
    )

    tiles = _tiles_of(caps)
    NT = len(tiles)
    offs = np.concatenate([[0], np.cumsum(caps)[:-1]]).astype(np.int64)
    out_dt = f16 if OUT_F16 else f32

    xt_d = nc.declare_dram_parameter("xt", [NT, 128, KC * TT], MM_DT, isOutput=False)
    w13_d = nc.declare_dram_parameter(
        "w13", [S, 4, 128, 2 * 2 * INTER], MM_DT, isOutput=False
    )
    w2_d = nc.declare_dram_parameter(
        "w2", [S, 128, JC * HIDDEN], MM_DT, isOutput=False
    )
    out_d = nc.declare_dram_parameter("out", [cap_total, HIDDEN], out_dt, isOutput=True)

    # SBUF pool sizing: slots scale with ESZ; keep total under ~23MB.
    big = ESZ == 4
    w13_bufs = 6 if big else 12
    w2_bufs = 2 if big else 3
    xt_bufs = 3 if big else 4
    hp_bufs = 12 if big else 18

    with tile.TileContext(nc) as tc:
        with (
            tc.tile_pool(name="w13p", bufs=w13_bufs) as w13p,
            tc.tile_pool(name="w2p", bufs=w2_bufs) as w2p,
            tc.tile_pool(name="xtp", bufs=xt_bufs) as xtp,
            tc.tile_pool(name="hp", bufs=hp_bufs) as hp,
            tc.tile_pool(name="sgp", bufs=6) as sgp,
            tc.tile_pool(name="outp", bufs=4) as outp,
            tc.tile_pool(name="ps1", bufs=4, space="PSUM") as ps1,
            tc.tile_pool(name="ps2", bufs=2, space="PSUM") as ps2,
        ):
            # No HAM warmup: the PE clock ramps (1.2GHz -> 2.4GHz after
            # ~3.4us sustained activity) during the opening wire-gated
            # weight staircase, where the PE is DMA-bound anyway — cold
            # matmuls there cost no end-time, and skipping warmup lets the
            # real stream start the moment pair0+xt land.

            # Deferred GEMM2 work queue: closures emitted between the next
            # segment's GEMM1 m-groups (fills LDWEIGHTS-bound PE stalls).
            g2q = []

            def pop_g2():
                if g2q:
                    g2q.pop(0)()

            # xt tiles, issued one segment ahead of use so the wire order is
            # [seg s weights][seg s+1 xt][seg s+1 weights]... and GEMM1 of
            # seg s+1 never waits on its activations.
            xt_tiles = {}
            seg_first_tile = {}
            for i, (ts, _, _) in enumerate(tiles):
                seg_first_tile.setdefault(ts, i)

            def issue_xt(i):
                if i in xt_tiles or i >= NT:
                    return
                _, _, tt_i = tiles[i]
                xtt = xtp.tile([128, KC * tt_i], MM_DT, tag="xtt",
                               name=f"xtt{i}", padded_shape=[128, KC * TT])
                nc.sync.dma_start(out=xtt[:], in_=xt_d[i, :, 0 : KC * tt_i])
                xt_tiles[i] = xtt

            tix = 0
            for s in range(S):
                C = caps[s]
                off = int(offs[s])

                # Weight DMAs in first-use (k) order; for the first segment
                # the first token tile's xt goes right after pair 0 so the
                # opening k-outer group consumes (pair0, xt, pair1, ...) as
                # they land on the wire.
                w13_t = []

                def load_pair(kp, s=s, w13_t=w13_t):
                    w13t = w13p.tile([128, 2 * 2 * INTER], MM_DT, tag="w13t",
                                     name=f"w13t{s}_{kp}")
                    nc.sync.dma_start(out=w13t[:], in_=w13_d[s, kp])
                    w13_t.append(w13t)

                if s == 0:
                    load_pair(0)
                    issue_xt(0)
                    for kp in range(1, 4):
                        load_pair(kp)
                else:
                    for kp in range(4):
                        load_pair(kp)
                w2t = w2p.tile([128, JC * HIDDEN], MM_DT, tag="w2t", name=f"w2t{s}")
                nc.sync.dma_start(out=w2t[:], in_=w2_d[s])
                # Remaining tiles of this segment, then prefetch the next
                # segment's first tile.
                for i in range(seg_first_tile[s], NT):
                    if tiles[i][0] != s:
                        break
                    issue_xt(i)
                if s + 1 < S:
                    issue_xt(seg_first_tile[s + 1])

                def w13_ap(k, m, w13_t=w13_t):
                    base = (k % 2) * 2 * INTER + 128 * m
                    return w13_t[k // 2][:, base : base + 128]

                def w2_ap(j, nn, w2t=w2t):
                    jw = min(128, INTER - 128 * j)
                    base = j * HIDDEN + 512 * nn
                    return w2t[0:jw, base : base + 512]

                for t0 in range(0, C, TT):
                    tt = min(TT, C - t0)
                    xt_tile = xt_tiles[tix]

                    def xt_ap(k, xt_tile=xt_tile, tt=tt):
                        return xt_tile[:, k * tt : (k + 1) * tt]

                    h_t = []
                    for j in range(JC):
                        jw = min(128, INTER - 128 * j)
                        ht = hp.tile([jw, tt], MM_DT, tag="ht", name=f"ht{tix}_{j}",
                                     padded_shape=[128, TT])
                        h_t.append(ht)

                    # GEMM1: k-outer within m-groups. The first token tile
                    # races the initial weight DMAs: one wide group of 8
                    # m-chunks (borrowing the two ps2 tiles as 4 psum-bank
                    # halves) gives the PE ~2.3us of matmul per arriving
                    # w13 k-pair (~1.8us wire each), hiding the staircase.
                    if tix == 0:
                        m_groups = [(0, 8), (8, MC)]
                    else:
                        m_groups = M_GROUPS
                    for m_lo, m_hi in m_groups:
                        # Pop a deferred GEMM2 chunk first: it is ready to
                        # run and keeps the PE fed while this group's first
                        # matmul may still be waiting on weights.
                        pop_g2()
                        pg_t = {}  # m -> (tile, col offset)
                        for m in range(m_lo, m_hi):
                            if m - m_lo < 4 or m_hi - m_lo <= 4:
                                pt = ps1.tile([128, tt], f32, tag="pg",
                                              name=f"pg{m}",
                                              padded_shape=[128, TT])
                                pg_t[m] = (pt, 0)
                            elif (m - m_lo) % 2 == 0:
                                bt = ps2.tile([128, 512 + tt], f32, tag="po",
                                              name=f"pgb{m}",
                                              padded_shape=[128, HIDDEN])
                                pg_t[m] = (bt, 0)
                            else:
                                pg_t[m] = (pg_t[m - 1][0], 512)

                        def pg_ap(m, r0=0, r1=128):
                            pt, c0 = pg_t[m]
                            return pt[r0:r1, c0 : c0 + tt]

                        for k in range(KC):
                            for m in range(m_lo, m_hi):
                                nc.tensor.matmul(
                                    pg_ap(m),
                                    w13_ap(k, m),
                                    xt_ap(k),
                                    start=(k == 0),
                                    stop=(k == KC - 1),
                                )
                        for m in range(m_lo, m_hi):
                            if m == MC - 1:
                                # tail chunk: [gate 64 | up 64] on partitions
                                sg = sgp.tile([64, tt], f32, tag="sg",
                                              name=f"sg{m}",
                                              padded_shape=[128, TT])
                                nc.scalar.activation(
                                    sg[:], pg_ap(m, 0, 64),
                                    mybir.ActivationFunctionType.Silu,
                                )
                                nc.vector.tensor_mul(
                                    h_t[JC - 1][0:64, :], sg[:],
                                    pg_ap(m, 64, 128),
                                )
                            elif m % 2 == 1:
                                # full pair: chunk m-1 = gate, chunk m = up
                                sg = sgp.tile([128, tt], f32, tag="sg",
                                              name=f"sg{m}",
                                              padded_shape=[128, TT])
                                nc.scalar.activation(
                                    sg[:], pg_ap(m - 1),
                                    mybir.ActivationFunctionType.Silu,
                                )
                                nc.vector.tensor_mul(
                                    h_t[m // 2][:], sg[:], pg_ap(m)
                                )

                    # GEMM2: h stationary, w2 moving; token-major output.
                    # Deferred: emitted between the NEXT segment's m-groups.
                    # The last segment (end of the program) runs nn-split:
                    # two [tw, 512] psum halves so the cast/store of half 0
                    # overlaps the matmuls of half 1, shortening the final
                    # dependency chain GEMM2 -> cast -> store.
                    def g2_chunk(tc0, tt=tt, h_t=h_t, w2_ap=w2_ap, off=off,
                                 t0=t0, s=s):
                        tw = min(128, tt - tc0)
                        row0 = off + t0 + tc0
                        store_eng = nc.sync if s == S - 1 else nc.gpsimd
                        if s == S - 1:
                            for nn in range(HIDDEN // 512):
                                po = ps2.tile([tw, 512], f32, tag="po",
                                              name="po", padded_shape=[128, HIDDEN])
                                for j in range(JC):
                                    nc.tensor.matmul(
                                        po[:],
                                        h_t[j][:, tc0 : tc0 + tw],
                                        w2_ap(j, nn),
                                        start=(j == 0),
                                        stop=(j == JC - 1),
                                    )
                                ob = outp.tile([tw, 512], out_dt, tag="ob",
                                               name="ob", padded_shape=[128, HIDDEN])
                                nc.vector.tensor_copy(ob[:], po[:])
                                store_eng.dma_start(
                                    out=out_d[row0 : row0 + tw,
                                              512 * nn : 512 * (nn + 1)],
                                    in_=ob[:],
                                )
                            return
                        po = ps2.tile([tw, HIDDEN], f32, tag="po", name="po",
                                      padded_shape=[128, HIDDEN])
                        for j in range(JC):
                            for nn in range(HIDDEN // 512):
                                nc.tensor.matmul(
                                    po[:, 512 * nn : 512 * (nn + 1)],
                                    h_t[j][:, tc0 : tc0 + tw],
                                    w2_ap(j, nn),
                                    start=(j == 0),
                                    stop=(j == JC - 1),
                                )
                        ob = outp.tile([tw, HIDDEN], out_dt, tag="ob", name="ob",
                                       padded_shape=[128, HIDDEN])
                        nc.vector.tensor_copy(ob[:], po[:])
                        # Stores ride the gpsimd (SWDGE) queue so they never
                        # block later loads on the sync sequencer; the last
                        # segment has no loads after it, so its stores take
                        # the faster HWDGE path.
                        store_eng.dma_start(
                            out=out_d[row0 : row0 + tw, :],
                            in_=ob[:],
                        )

                    for tc0 in range(0, tt, 128):
                        g2q.append(lambda tc0=tc0: g2_chunk(tc0))
                    tix += 1

            while g2q:
                pop_g2()

    nc.compile()
    return nc


_BUILD_CACHE = {}


def _get_program(S, caps, cap_total):
    key = (S, tuple(caps), str(MM_DT), OUT_F16)
    if key not in _BUILD_CACHE:
        _BUILD_CACHE[key] = _build(S, caps, cap_total)
    return _BUILD_CACHE[key]


def _pack_inputs(x, assign, caps, offs, cap_total, w13_perm, w2):
    """Build per-core input dicts matching the device layouts."""
    tiles = _tiles_of(caps)
    NT = len(tiles)
    S = len(caps)
    in_maps = []
    for c in range(NCORES):
        xt_c = np.zeros((HIDDEN, cap_total), dtype=NP_DT)
        w13_c = np.zeros((S, 4, 128, 2 * 2 * INTER), dtype=NP_DT)
        w2_c = np.zeros((S, 128, JC * HIDDEN), dtype=NP_DT)
        for s, (e, a, n) in enumerate(assign[c]):
            if e is None or n <= 0:
                continue
            o = int(offs[s])
            xt_c[:, o : o + n] = _to_np_dt(x[a : a + n, :]).T
            # w13: [1024, 1408] -> [4, 2, 128, 1408] -> [4, 128, 2*1408]
            w13_c[s] = (
                w13_perm["w13"][e]
                .reshape(4, 2, 128, 2 * INTER)
                .transpose(0, 2, 1, 3)
                .reshape(4, 128, 2 * 2 * INTER)
            )
            # w2: pad [704,1024] -> [768,1024] -> [6,128,1024] -> [128, 6*1024]
            w2_c[s] = w13_perm["w2"][e]
        # xt: per token tile [1024, tt] -> [8, 128, tt] -> [128, 8*tt]
        xt_pack = np.zeros((NT, 128, KC * TT), dtype=NP_DT)
        for tix, (s, t0, tt) in enumerate(tiles):
            o = int(offs[s])
            blk = xt_c[:, o + t0 : o + t0 + tt]  # [1024, tt]
            xt_pack[tix, :, 0 : KC * tt] = (
                blk.reshape(KC, 128, tt).transpose(1, 0, 2).reshape(128, KC * tt)
            )
        in_maps.append({"xt": xt_pack, "w13": w13_c, "w2": w2_c})
    return in_maps


def _prep_weights(w1w3, w2):
    """Permute/pack weights once (shared across cores)."""
    w13_perm = _to_np_dt(w1w3[:, :, _PERM])  # [E, HIDDEN, 2*INTER]
    w2p_all = np.zeros((N_EXPERTS, 768, HIDDEN), dtype=NP_DT)
    w2p_all[:, :INTER] = _to_np_dt(w2)
    w2_pack = (
        w2p_all.reshape(N_EXPERTS, JC, 128, HIDDEN)
        .transpose(0, 2, 1, 3)
        .reshape(N_EXPERTS, 128, JC * HIDDEN)
    )
    return {"w13": w13_perm, "w2": w2_pack}


def _run(x, tokens_per_expert, w1w3, w2, trace=False):
    x = np.ascontiguousarray(np.asarray(x, dtype=np.float32))
    counts = np.asarray(tokens_per_expert, dtype=np.int64).copy()
    w1w3 = np.asarray(w1w3, dtype=np.float32)
    w2 = np.asarray(w2, dtype=np.float32)

    T = x.shape[0]
    # Clip group sizes like ragged_dot: groups are consecutive; anything
    # beyond T is out of range.
    counts = np.maximum(counts, 0)
    cum = np.cumsum(counts)
    over = cum > T
    if over.any():
        first = int(np.argmax(over))
        prev = int(cum[first - 1]) if first > 0 else 0
        counts[first] = T - prev
        counts[first + 1 :] = 0

    assign, caps, offs, cap_total = _plan(counts)
    S = len(caps)
    nc = _get_program(S, caps, cap_total)

    packed_w = _prep_weights(w1w3, w2)
    in_maps = _pack_inputs(x, assign, caps, offs, cap_total, packed_w, w2)

    extra = {}
    if trace:
        import os

        os.makedirs("/tmp/moe_prof", exist_ok=True)
        for f in os.listdir("/tmp/moe_prof"):
            os.unlink(os.path.join("/tmp/moe_prof", f))
        extra["tmpdir"] = "/tmp/moe_prof"
    res = run_bass_kernel_spmd(nc, in_maps, list(range(NCORES)), trace=trace, **extra)

    out_full = np.zeros((T, HIDDEN), dtype=np.float32)
    for c in range(NCORES):
        oc = res.results[c]["out"]
        for s, (e, a, n) in enumerate(assign[c]):
            if e is None or n <= 0:
                continue
            o = int(offs[s])
            out_full[a : a + n, :] = oc[o : o + n, :].astype(np.float32)
    return out_full, res


def kernel(x, tokens_per_expert, w1w3, w2, decoding=False, **_ignored):
    out, _ = _run(x, tokens_per_expert, w1w3, w2, trace=False)
    return out


# revision 15
# speedup vs baseline: 1.2150x; 1.2150x over previous
"""MoE grouped-GEMM expert FFN (SwiGLU) for Trainium2, 8-core expert parallelism.

Contract: kernel(**inputs) takes FULL unsharded inputs, returns FULL output.

Strategy:
  - Host-side routing: tokens are contiguous per expert; split expert groups
    into chunks, band-assign chunks across 8 cores with an identical
    segment-capacity structure on every core (SPMD: one Bass program).
  - Per core, per segment: local GEMM1 (x @ w1w3) -> SwiGLU -> GEMM2 (h @ w2).
  - Host-side combine: scatter per-core output rows back to full output.

Matmul dtype is configurable (MM_DT): float32r runs at full PE rate with
~2.5e-4 rel err; float16 additionally halves DMA bytes and enables fast
weight load, at ~1e-3 rel err. PSUM/silu stay fp32; the output is stored
fp16 (quantization ~2.4e-4 of absmax) and cast back to fp32 on host.

Layout choices:
  - All device inputs are host-repacked so every DMA loads long contiguous
    rows with few instructions (DMA issue costs ~0.6-1.3us per instruction
    on the sync sequencer; per-engine DMA bandwidth scales with run length).
  - x: packed per token tile as [tile, 128, 8*512] (hidden chunk k on the
    free dim) -> 1 DMA per token tile.
  - w1w3: columns permuted so psum chunk c holds gate[64c:64c+64] on
    partitions 0:64 and up on 64:128 (SwiGLU = partition-slice op); rows
    packed as [S, 4, 128, 2*1408] (k-chunk pairs) -> 4 DMAs per segment.
  - w2: rows packed as [S, 128, 6*1024] (j on free dim; j=5 has 64 valid
    rows) -> 1 DMA per segment.
  - GEMM1 iterates k (contraction) outer / m inner within m-groups of <=4 so
    compute starts after ~1MB of DMA and segment boundaries pipeline.
  - GEMM2 uses h as stationary ([inter, token] slices) and w2 as moving ->
    output lands token-major in PSUM and stores contiguously (via gpsimd
    queue to keep the sync sequencer free for loads).
  - GEMM2 of segment s is deferred and interleaved between GEMM1 m-groups
    of segment s+1: small segments' GEMM1 is LDWEIGHTS-bound (88 loads of
    ~88ns vs <90ns of streaming each), and the interleaved GEMM2 matmuls
    (218ns streams) keep the PE busy while loads are pulled ahead into the
    background weight buffer.
"""

import numpy as np

import concourse.bacc as bacc
import concourse.mybir as mybir
from concourse import tile
from concourse.bass_utils import run_bass_kernel_spmd

HIDDEN = 1024
INTER = 704
N_EXPERTS = 32
NCORES = 8
KC = HIDDEN // 128  # 8 k-chunks over hidden
MC = (2 * INTER) // 128  # 11 m-chunks over permuted gate|up dim
JC = (INTER + 127) // 128  # 6 j-chunks over inter for GEMM2 (last is 64 rows)
TT = 512  # token tile (moving free dim)
M_GROUPS = [(0, 2), (2, 4), (4, 6), (6, 8), (8, 10), (10, 11)]  # pair-sized m-groups

f32 = mybir.dt.float32
f16 = mybir.dt.float16

# Matmul input dtype. float16 runs ~1.7x faster than float32r at ~4.6e-4
# rel err (vs 2.5e-4 for f32r); PSUM accumulation is fp32 either way.
MM_DT = mybir.dt.float16
NP_DT = np.float16
ESZ = 2  # element size of MM_DT in bytes
OUT_F16 = True  # store output fp16 (halves store DMA), cast to fp32 on host


def set_dtype(name):
    global MM_DT, NP_DT, ESZ
    if name == "f32r":
        MM_DT, NP_DT, ESZ = mybir.dt.float32r, np.float32, 4
    elif name == "f16":
        MM_DT, NP_DT, ESZ = mybir.dt.float16, np.float16, 2
    elif name == "bf16":
        MM_DT, NP_DT, ESZ = mybir.dt.bfloat16, np.float32, 2  # cast via jax-free trick
    else:
        raise ValueError(name)


# Column permutation of w1w3's last dim (2*INTER): m-chunks come in
# (gate, up) pairs of full 128-row blocks so SwiGLU runs full-width
# [128, tt] ACT/DVE ops (engine cost scales with free size, not partition
# count). chunk 2j = gate[128j:128j+128], chunk 2j+1 = up[128j:128j+128]
# for j<5; the last chunk holds the 64-row tails [gate[640:704]|up[640:704]].
_PERM = np.empty(2 * INTER, dtype=np.int64)
for _j in range(5):
    _PERM[256 * _j : 256 * _j + 128] = np.arange(128 * _j, 128 * _j + 128)
    _PERM[256 * _j + 128 : 256 * _j + 256] = INTER + np.arange(
        128 * _j, 128 * _j + 128
    )
_PERM[1280:1344] = np.arange(640, 704)
_PERM[1344:1408] = INTER + np.arange(640, 704)


def _to_np_dt(a):
    """Cast fp32 array to the host dtype for MM_DT."""
    if MM_DT == mybir.dt.bfloat16:
        b = np.asarray(a, dtype=np.float32).copy()
        v = b.view(np.uint32)
        v += 0x8000  # round-to-nearest-even-ish
        v &= 0xFFFF0000
        return b
    return np.asarray(a, dtype=NP_DT)


def _make_chunks(counts, starts, tmax):
    chunks = []  # (n, expert, tok_start)
    for e in range(N_EXPERTS):
        n = int(counts[e])
        a = int(starts[e])
        if n <= 0:
            continue
        nparts = -(-n // tmax)
        base, rem = divmod(n, nparts)
        off = 0
        for p in range(nparts):
            ln = base + (1 if p < rem else 0)
            if ln > 0:
                chunks.append((ln, e, a + off))
                off += ln
    return chunks


def _pe_time(caps):
    """Predicted PE critical time for a segment-cap structure (fp16)."""
    col = 0.427e-9  # per moving column at 2.4GHz
    ld = 88e-9  # LDWEIGHTS (measured on HW, FWL fp16)
    t = 0.0
    for C in caps:
        for t0 in range(0, C, TT):
            tt = min(TT, C - t0)
            t += 88 * max(tt * col, ld)  # GEMM1: 8k x 11m stationary loads
            t += -(-tt // 128) * 12 * max(512 * col, ld)  # GEMM2 chunks
    return t


def _dma_time(S, cap_total):
    w_seg = (HIDDEN * 2 * INTER + 768 * HIDDEN) * ESZ  # w13 + padded w2
    out_b = 2 if OUT_F16 else 4
    return (S * w_seg + cap_total * HIDDEN * (ESZ + out_b)) / 390e9


def _plan(counts):
    """Balance (expert, token-chunk) pieces across NCORES cores.

    Chunks are sorted by size and dealt in bands of 8 (one per core): slot s
    capacity = the largest chunk in band s, which minimizes total capacity
    for a given chunk multiset. The split threshold trades segment count
    (weight DMA traffic) against padding (PE + activation traffic).
    """
    starts = np.zeros(N_EXPERTS, dtype=np.int64)
    np.cumsum(counts[:-1], out=starts[1:])

    best = None
    for tmax in (4096, 2048, 1536, 1024, *range(256, 1025, 16)):
        chunks = _make_chunks(counts, starts, max(1, tmax))
        if not chunks:
            chunks = [(0, None, 0)]
        chunks.sort(key=lambda c: -c[0])
        S = -(-len(chunks) // NCORES)
        caps = []
        for s in range(S):
            band = chunks[NCORES * s : NCORES * (s + 1)]
            caps.append(max(8, ((band[0][0] + 7) // 8) * 8))
        cap_total = sum(caps)
        dma_t = _dma_time(S, cap_total)
        pe_t = _pe_time(caps)
        score = max(dma_t, pe_t) + 0.2 * min(dma_t, pe_t)
        if best is None or score < best[0]:
            best = (score, chunks, S, caps)

    _, chunks, S, caps = best
    offs = np.concatenate([[0], np.cumsum(caps)[:-1]]).astype(np.int64)
    cap_total = int(sum(caps))

    assign = [[] for _ in range(NCORES)]
    for s in range(S):
        band = chunks[NCORES * s : NCORES * (s + 1)]
        for c in range(NCORES):
            if c < len(band):
                n, e, a = band[c]
                assign[c].append((e, a, n))
            else:
                assign[c].append((None, 0, 0))
    return assign, caps, offs, cap_total


def _tiles_of(caps):
    """Token tiles as (segment, t0, tt) in execution order."""
    out = []
    for s, C in enumerate(caps):
        for t0 in range(0, C, TT):
            out.append((s, t0, min(TT, C - t0)))
    return out


def _build(S, caps, cap_total):
    """Build the SPMD Bass program for one core's segment structure."""
    nc = bacc.Bacc(
        "TRN2",
        target_bir_lowering=False,
        debug=False,
        enable_asserts=False,
        num_devices=NCORES,
    )

    tiles = _tiles_of(caps)
    NT = len(tiles)
    offs = np.concatenate([[0], np.cumsum(caps)[:-1]]).astype(np.int64)
    out_dt = f16 if OUT_F16 else f32

    xt_d = nc.declare_dram_parameter("xt", [NT, 128, KC * TT], MM_DT, isOutput=False)
    w13_d = nc.declare_dram_parameter(
        "w13", [S, 4, 128, 2 * 2 * INTER], MM_DT, isOutput=False
    )
    w2_d = nc.declare_dram_parameter(
        "w2", [S, 128, JC * HIDDEN], MM_DT, isOutput=False
    )
    out_d = nc.declare_dram_parameter("out", [cap_total, HIDDEN], out_dt, isOutput=True)

    # SBUF pool sizing: slots scale with ESZ; keep total under ~23MB.
    big = ESZ == 4
    w13_bufs = 6 if big else 12
    w2_bufs = 2 if big else 3
    xt_bufs = 3 if big else 4
    hp_bufs = 12 if big else 18

    with tile.TileContext(nc) as tc:
        with (
            tc.tile_pool(name="w13p", bufs=w13_bufs) as w13p,
            tc.tile_pool(name="w2p", bufs=w2_bufs) as w2p,
            tc.tile_pool(name="xtp", bufs=xt_bufs) as xtp,
            tc.tile_pool(name="hp", bufs=hp_bufs) as hp,
            tc.tile_pool(name="sgp", bufs=6) as sgp,
            tc.tile_pool(name="outp", bufs=4) as outp,
            tc.tile_pool(name="ps1", bufs=4, space="PSUM") as ps1,
            tc.tile_pool(name="ps2", bufs=2, space="PSUM") as ps2,
        ):
            # No HAM warmup: the PE clock ramps (1.2GHz -> 2.4GHz after
            # ~3.4us sustained activity) during the opening wire-gated
            # weight staircase, where the PE is DMA-bound anyway — cold
            # matmuls there cost no end-time, and skipping warmup lets the
            # real stream start the moment pair0+xt land.

            # Deferred GEMM2 work queue: closures emitted between the next
            # segment's GEMM1 m-groups (fills LDWEIGHTS-bound PE stalls).
            g2q = []

            def pop_g2():
                if g2q:
                    g2q.pop(0)()

            # xt tiles, issued one segment ahead of use so the wire order is
            # [seg s weights][seg s+1 xt][seg s+1 weights]... and GEMM1 of
            # seg s+1 never waits on its activations.
            xt_tiles = {}
            seg_first_tile = {}
            for i, (ts, _, _) in enumerate(tiles):
                seg_first_tile.setdefault(ts, i)

            def issue_xt(i):
                if i in xt_tiles or i >= NT:
                    return
                _, _, tt_i = tiles[i]
                xtt = xtp.tile([128, KC * tt_i], MM_DT, tag="xtt",
                               name=f"xtt{i}", padded_shape=[128, KC * TT])
                nc.sync.dma_start(out=xtt[:], in_=xt_d[i, :, 0 : KC * tt_i])
                xt_tiles[i] = xtt

            tix = 0
            for s in range(S):
                C = caps[s]
                off = int(offs[s])

                # Weight DMAs in first-use (k) order; for the first segment
                # the first token tile's xt goes right after pair 0 so the
                # opening k-outer group consumes (pair0, xt, pair1, ...) as
                # they land on the wire.
                w13_t = []

                def load_pair(kp, s=s, w13_t=w13_t):
                    w13t = w13p.tile([128, 2 * 2 * INTER], MM_DT, tag="w13t",
                                     name=f"w13t{s}_{kp}")
                    nc.sync.dma_start(out=w13t[:], in_=w13_d[s, kp])
                    w13_t.append(w13t)

                if s == 0:
                    # Startup race: split pair0 and xt0 into k-halves so the
                    # opening k-loop's first matmuls start after ~0.7MB on
                    # the wire instead of ~1.4MB. Disjoint-slice DMAs into
                    # one tile: readers of the k=0 half wait only on DMA 0
                    # (view-overlap hazard tracking is slice-granular).
                    w13t0 = w13p.tile([128, 2 * 2 * INTER], MM_DT,
                                      tag="w13t", name="w13t0_0")
                    nc.sync.dma_start(out=w13t0[:, 0 : 2 * INTER],
                                      in_=w13_d[0, 0, :, 0 : 2 * INTER])
                    w13_t.append(w13t0)
                    tt0 = tiles[0][2]
                    xh = (KC // 2) * tt0
                    xtt0 = xtp.tile([128, KC * tt0], MM_DT, tag="xtt",
                                    name="xtt0", padded_shape=[128, KC * TT])
                    nc.sync.dma_start(out=xtt0[:, 0:xh], in_=xt_d[0, :, 0:xh])
                    xt_tiles[0] = xtt0
                    nc.sync.dma_start(
                        out=w13t0[:, 2 * INTER : 4 * INTER],
                        in_=w13_d[0, 0, :, 2 * INTER : 4 * INTER],
                    )
                    load_pair(1)
                    nc.sync.dma_start(out=xtt0[:, xh : KC * tt0],
                                      in_=xt_d[0, :, xh : KC * tt0])
                    load_pair(2)
                    load_pair(3)
                else:
                    for kp in range(4):
                        load_pair(kp)
                w2t = w2p.tile([128, JC * HIDDEN], MM_DT, tag="w2t", name=f"w2t{s}")
                nc.sync.dma_start(out=w2t[:], in_=w2_d[s])
                # Remaining tiles of this segment, then prefetch the next
                # segment's first tile.
                for i in range(seg_first_tile[s], NT):
                    if tiles[i][0] != s:
                        break
                    issue_xt(i)
                if s + 1 < S:
                    issue_xt(seg_first_tile[s + 1])

                def w13_ap(k, m, w13_t=w13_t):
                    base = (k % 2) * 2 * INTER + 128 * m
                    return w13_t[k // 2][:, base : base + 128]

                def w2_ap(j, nn, w2t=w2t):
                    # Full 128 rows even for the j=5 tail: rows 704:768 are
                    # host-packed zeros and h's pad rows are memset to zero,
                    # so the matmul result is unchanged but the stationary
                    # is 128-wide (FWL eligible).
                    base = j * HIDDEN + 512 * nn
                    return w2t[:, base : base + 512]

                for t0 in range(0, C, TT):
                    tt = min(TT, C - t0)
                    xt_tile = xt_tiles[tix]

                    def xt_ap(k, xt_tile=xt_tile, tt=tt):
                        return xt_tile[:, k * tt : (k + 1) * tt]

                    h_t = []
                    for j in range(JC):
                        jw = min(128, INTER - 128 * j)
                        if jw < 128:
                            # Allocate the tail h chunk full-height and zero
                            # the pad rows: GEMM2 can then use a 128-column
                            # stationary (FWL eligible, ~50ns faster per
                            # matmul); the pad rows hit w2's zero pad rows.
                            ht = hp.tile([128, tt], MM_DT, tag="ht",
                                         name=f"ht{tix}_{j}",
                                         padded_shape=[128, TT])
                            nc.vector.memset(ht[jw:128, :], 0.0)
                        else:
                            ht = hp.tile([jw, tt], MM_DT, tag="ht",
                                         name=f"ht{tix}_{j}",
                                         padded_shape=[128, TT])
                        h_t.append(ht)

                    # GEMM1: k-outer within m-groups. The first token tile
                    # races the initial weight DMAs: one wide group of 8
                    # m-chunks (borrowing the two ps2 tiles as 4 psum-bank
                    # halves) gives the PE ~2.3us of matmul per arriving
                    # w13 k-pair (~1.8us wire each), hiding the staircase.
                    if tix == 0:
                        m_groups = [(0, 8), (8, MC)]
                    else:
                        m_groups = M_GROUPS
                    for m_lo, m_hi in m_groups:
                        # Pop a deferred GEMM2 chunk first: it is ready to
                        # run and keeps the PE fed while this group's first
                        # matmul may still be waiting on weights.
                        pop_g2()
                        pg_t = {}  # m -> (tile, col offset)
                        for m in range(m_lo, m_hi):
                            if m - m_lo < 4 or m_hi - m_lo <= 4:
                                pt = ps1.tile([128, tt], f32, tag="pg",
                                              name=f"pg{m}",
                                              padded_shape=[128, TT])
                                pg_t[m] = (pt, 0)
                            elif (m - m_lo) % 2 == 0:
                                bt = ps2.tile([128, 512 + tt], f32, tag="po",
                                              name=f"pgb{m}",
                                              padded_shape=[128, HIDDEN])
                                pg_t[m] = (bt, 0)
                            else:
                                pg_t[m] = (pg_t[m - 1][0], 512)

                        def pg_ap(m, r0=0, r1=128):
                            pt, c0 = pg_t[m]
                            return pt[r0:r1, c0 : c0 + tt]

                        for k in range(KC):
                            for m in range(m_lo, m_hi):
                                nc.tensor.matmul(
                                    pg_ap(m),
                                    w13_ap(k, m),
                                    xt_ap(k),
                                    start=(k == 0),
                                    stop=(k == KC - 1),
                                )
                        for m in range(m_lo, m_hi):
                            if m == MC - 1:
                                # tail chunk: [gate 64 | up 64] on partitions
                                sg = sgp.tile([64, tt], f32, tag="sg",
                                              name=f"sg{m}",
                                              padded_shape=[128, TT])
                                nc.scalar.activation(
                                    sg[:], pg_ap(m, 0, 64),
                                    mybir.ActivationFunctionType.Silu,
                                )
                                nc.vector.tensor_mul(
                                    h_t[JC - 1][0:64, :], sg[:],
                                    pg_ap(m, 64, 128),
                                )
                            elif m % 2 == 1:
                                # full pair: chunk m-1 = gate, chunk m = up
                                sg = sgp.tile([128, tt], f32, tag="sg",
                                              name=f"sg{m}",
                                              padded_shape=[128, TT])
                                nc.scalar.activation(
                                    sg[:], pg_ap(m - 1),
                                    mybir.ActivationFunctionType.Silu,
                                )
                                nc.vector.tensor_mul(
                                    h_t[m // 2][:], sg[:], pg_ap(m)
                                )

                    # GEMM2: h stationary, w2 moving; token-major output.
                    # Deferred: emitted between the NEXT segment's m-groups.
                    # The last segment (end of the program) runs nn-split:
                    # two [tw, 512] psum halves so the cast/store of half 0
                    # overlaps the matmuls of half 1, shortening the final
                    # dependency chain GEMM2 -> cast -> store.
                    def g2_chunk(tc0, tt=tt, h_t=h_t, w2_ap=w2_ap, off=off,
                                 t0=t0, s=s):
                        tw = min(128, tt - tc0)
                        row0 = off + t0 + tc0
                        store_eng = nc.sync if s == S - 1 else nc.gpsimd
                        if s == S - 1:
                            for nn in range(HIDDEN // 512):
                                po = ps2.tile([tw, 512], f32, tag="po",
                                              name="po", padded_shape=[128, HIDDEN])
                                for j in range(JC):
                                    nc.tensor.matmul(
                                        po[:],
                                        h_t[j][:, tc0 : tc0 + tw],
                                        w2_ap(j, nn),
                                        start=(j == 0),
                                        stop=(j == JC - 1),
                                    )
                                ob = outp.tile([tw, 512], out_dt, tag="ob",
                                               name="ob", padded_shape=[128, HIDDEN])
                                nc.vector.tensor_copy(ob[:], po[:])
                                store_eng.dma_start(
                                    out=out_d[row0 : row0 + tw,
                                              512 * nn : 512 * (nn + 1)],
                                    in_=ob[:],
                                )
                            return
                        po = ps2.tile([tw, HIDDEN], f32, tag="po", name="po",
                                      padded_shape=[128, HIDDEN])
                        for j in range(JC):
                            for nn in range(HIDDEN // 512):
                                nc.tensor.matmul(
                                    po[:, 512 * nn : 512 * (nn + 1)],
                                    h_t[j][:, tc0 : tc0 + tw],
                                    w2_ap(j, nn),
                                    start=(j == 0),
                                    stop=(j == JC - 1),
                                )
                        ob = outp.tile([tw, HIDDEN], out_dt, tag="ob", name="ob",
                                       padded_shape=[128, HIDDEN])
                        nc.vector.tensor_copy(ob[:], po[:])
                        # Stores ride the gpsimd (SWDGE) queue so they never
                        # block later loads on the sync sequencer; the last
                        # segment has no loads after it, so its stores take
                        # the faster HWDGE path.
                        store_eng.dma_start(
                            out=out_d[row0 : row0 + tw, :],
                            in_=ob[:],
                        )

                    for tc0 in range(0, tt, 128):
                        g2q.append(lambda tc0=tc0: g2_chunk(tc0))
                    tix += 1

            while g2q:
                pop_g2()

    nc.compile()
    return nc


_BUILD_CACHE = {}


def _get_program(S, caps, cap_total):
    key = (S, tuple(caps), str(MM_DT), OUT_F16)
    if key not in _BUILD_CACHE:
        _BUILD_CACHE[key] = _build(S, caps, cap_total)
    return _BUILD_CACHE[key]


def _pack_inputs(x, assign, caps, offs, cap_total, w13_perm, w2):
    """Build per-core input dicts matching the device layouts."""
    tiles = _tiles_of(caps)
    NT = len(tiles)
    S = len(caps)
    in_maps = []
    for c in range(NCORES):
        xt_c = np.zeros((HIDDEN, cap_total), dtype=NP_DT)
        w13_c = np.zeros((S, 4, 128, 2 * 2 * INTER), dtype=NP_DT)
        w2_c = np.zeros((S, 128, JC * HIDDEN), dtype=NP_DT)
        for s, (e, a, n) in enumerate(assign[c]):
            if e is None or n <= 0:
                continue
            o = int(offs[s])
            xt_c[:, o : o + n] = _to_np_dt(x[a : a + n, :]).T
            # w13: [1024, 1408] -> [4, 2, 128, 1408] -> [4, 128, 2*1408]
            w13_c[s] = (
                w13_perm["w13"][e]
                .reshape(4, 2, 128, 2 * INTER)
                .transpose(0, 2, 1, 3)
                .reshape(4, 128, 2 * 2 * INTER)
            )
            # w2: pad [704,1024] -> [768,1024] -> [6,128,1024] -> [128, 6*1024]
            w2_c[s] = w13_perm["w2"][e]
        # xt: per token tile [1024, tt] -> [8, 128, tt] -> [128, 8*tt]
        xt_pack = np.zeros((NT, 128, KC * TT), dtype=NP_DT)
        for tix, (s, t0, tt) in enumerate(tiles):
            o = int(offs[s])
            blk = xt_c[:, o + t0 : o + t0 + tt]  # [1024, tt]
            xt_pack[tix, :, 0 : KC * tt] = (
                blk.reshape(KC, 128, tt).transpose(1, 0, 2).reshape(128, KC * tt)
            )
        in_maps.append({"xt": xt_pack, "w13": w13_c, "w2": w2_c})
    return in_maps


def _prep_weights(w1w3, w2):
    """Permute/pack weights once (shared across cores)."""
    w13_perm = _to_np_dt(w1w3[:, :, _PERM])  # [E, HIDDEN, 2*INTER]
    w2p_all = np.zeros((N_EXPERTS, 768, HIDDEN), dtype=NP_DT)
    w2p_all[:, :INTER] = _to_np_dt(w2)
    w2_pack = (
        w2p_all.reshape(N_EXPERTS, JC, 128, HIDDEN)
        .transpose(0, 2, 1, 3)
        .reshape(N_EXPERTS, 128, JC * HIDDEN)
    )
    return {"w13": w13_perm, "w2": w2_pack}


def _run(x, tokens_per_expert, w1w3, w2, trace=False):
    x = np.ascontiguousarray(np.asarray(x, dtype=np.float32))
    counts = np.asarray(tokens_per_expert, dtype=np.int64).copy()
    w1w3 = np.asarray(w1w3, dtype=np.float32)
    w2 = np.asarray(w2, dtype=np.float32)

    T = x.shape[0]
    # Clip group sizes like ragged_dot: groups are consecutive; anything
    # beyond T is out of range.
    counts = np.maximum(counts, 0)
    cum = np.cumsum(counts)
    over = cum > T
    if over.any():
        first = int(np.argmax(over))
        prev = int(cum[first - 1]) if first > 0 else 0
        counts[first] = T - prev
        counts[first + 1 :] = 0

    assign, caps, offs, cap_total = _plan(counts)
    S = len(caps)
    nc = _get_program(S, caps, cap_total)

    packed_w = _prep_weights(w1w3, w2)
    in_maps = _pack_inputs(x, assign, caps, offs, cap_total, packed_w, w2)

    extra = {}
    if trace:
        import os

        os.makedirs("/tmp/moe_prof", exist_ok=True)
        for f in os.listdir("/tmp/moe_prof"):
            os.unlink(os.path.join("/tmp/moe_prof", f))
        extra["tmpdir"] = "/tmp/moe_prof"
    res = run_bass_kernel_spmd(nc, in_maps, list(range(NCORES)), trace=trace, **extra)

    out_full = np.zeros((T, HIDDEN), dtype=np.float32)
    for c in range(NCORES):
        oc = res.results[c]["out"]
        for s, (e, a, n) in enumerate(assign[c]):
            if e is None or n <= 0:
                continue
            o = int(offs[s])
            out_full[a : a + n, :] = oc[o : o + n, :].astype(np.float32)
    return out_full, res


def kernel(x, tokens_per_expert, w1w3, w2, decoding=False, **_ignored):
    out, _ = _run(x, tokens_per_expert, w1w3, w2, trace=False)
    return out


# revision 17
# speedup vs baseline: 1.2190x; 1.0033x over previous
"""MoE grouped-GEMM expert FFN (SwiGLU) for Trainium2, 8-core expert parallelism.

Contract: kernel(**inputs) takes FULL unsharded inputs, returns FULL output.

Strategy:
  - Host-side routing: tokens are contiguous per expert; split expert groups
    into chunks, band-assign chunks across 8 cores with an identical
    segment-capacity structure on every core (SPMD: one Bass program).
  - Per core, per segment: local GEMM1 (x @ w1w3) -> SwiGLU -> GEMM2 (h @ w2).
  - Host-side combine: scatter per-core output rows back to full output.

Matmul dtype is configurable (MM_DT): float32r runs at full PE rate with
~2.5e-4 rel err; float16 additionally halves DMA bytes and enables fast
weight load, at ~1e-3 rel err. PSUM/silu stay fp32; the output is stored
fp16 (quantization ~2.4e-4 of absmax) and cast back to fp32 on host.

Layout choices:
  - All device inputs are host-repacked so every DMA loads long contiguous
    rows with few instructions (DMA issue costs ~0.6-1.3us per instruction
    on the sync sequencer; per-engine DMA bandwidth scales with run length).
  - x: packed per token tile as [tile, 128, 8*512] (hidden chunk k on the
    free dim) -> 1 DMA per token tile.
  - w1w3: columns permuted so psum chunk c holds gate[64c:64c+64] on
    partitions 0:64 and up on 64:128 (SwiGLU = partition-slice op); rows
    packed as [S, 4, 128, 2*1408] (k-chunk pairs) -> 4 DMAs per segment.
  - w2: rows packed as [S, 128, 6*1024] (j on free dim; j=5 has 64 valid
    rows) -> 1 DMA per segment.
  - GEMM1 iterates k (contraction) outer / m inner within m-groups of 2
    psum tiles; segment 0's first tile uses one wide group of 8 m-chunks
    (4 ps1 tiles + both ps2 tiles as bank halves) so the opening k-loop
    does ~2.3us of matmul per arriving w13 k-pair (~1.8us wire each) and
    rides the initial weight staircase without idling.
  - No HAM warmup: the PE clock ramp (1.2->2.4GHz after ~3.4us sustained)
    happens during the wire-gated staircase where cold matmuls cost no
    end-time.
  - Startup DMAs are k-granular (pair0 and xt0 split in half) so the first
    matmul starts after ~0.7MB on the wire instead of ~1.4MB.
  - GEMM2 uses h as stationary ([inter, token] slices) and w2 as moving ->
    output lands token-major in PSUM and stores contiguously (via gpsimd
    queue to keep the sync sequencer free for loads). The j=5 tail chunk
    is zero-padded to 128 rows (against w2's zero pad rows) to stay FWL
    eligible (~50ns/matmul faster).
  - GEMM2 of segment s is deferred and emitted BEFORE each GEMM1 m-group
    of segment s+1: ready GEMM2 streams sit ahead of weight-waiting GEMM1
    matmuls in the PE FIFO, covering DMA staircases and LDWEIGHTS-bound
    stretches. The last segment's GEMM2 runs nn-split ([tw,512] halves)
    to shorten the final matmul->cast->store chain.
  - xt tiles are prefetched one segment ahead so GEMM1 never waits on
    activations; output is stored fp16 and cast to fp32 on host.
"""

import numpy as np

import concourse.bacc as bacc
import concourse.mybir as mybir
from concourse import tile
from concourse.bass_utils import run_bass_kernel_spmd

HIDDEN = 1024
INTER = 704
N_EXPERTS = 32
NCORES = 8
KC = HIDDEN // 128  # 8 k-chunks over hidden
MC = (2 * INTER) // 128  # 11 m-chunks over permuted gate|up dim
JC = (INTER + 127) // 128  # 6 j-chunks over inter for GEMM2 (last is 64 rows)
TT = 512  # token tile (moving free dim)
M_GROUPS = [(0, 2), (2, 4), (4, 6), (6, 8), (8, 10), (10, 11)]  # pair-sized m-groups

f32 = mybir.dt.float32
f16 = mybir.dt.float16

# Matmul input dtype. float16 runs ~1.7x faster than float32r at ~4.6e-4
# rel err (vs 2.5e-4 for f32r); PSUM accumulation is fp32 either way.
MM_DT = mybir.dt.float16
NP_DT = np.float16
ESZ = 2  # element size of MM_DT in bytes
OUT_F16 = True  # store output fp16 (halves store DMA), cast to fp32 on host


def set_dtype(name):
    global MM_DT, NP_DT, ESZ
    if name == "f32r":
        MM_DT, NP_DT, ESZ = mybir.dt.float32r, np.float32, 4
    elif name == "f16":
        MM_DT, NP_DT, ESZ = mybir.dt.float16, np.float16, 2
    elif name == "bf16":
        MM_DT, NP_DT, ESZ = mybir.dt.bfloat16, np.float32, 2  # cast via jax-free trick
    else:
        raise ValueError(name)


# Column permutation of w1w3's last dim (2*INTER): m-chunks come in
# (gate, up) pairs of full 128-row blocks so SwiGLU runs full-width
# [128, tt] ACT/DVE ops (engine cost scales with free size, not partition
# count). chunk 2j = gate[128j:128j+128], chunk 2j+1 = up[128j:128j+128]
# for j<5; the last chunk holds the 64-row tails [gate[640:704]|up[640:704]].
_PERM = np.empty(2 * INTER, dtype=np.int64)
for _j in range(5):
    _PERM[256 * _j : 256 * _j + 128] = np.arange(128 * _j, 128 * _j + 128)
    _PERM[256 * _j + 128 : 256 * _j + 256] = INTER + np.arange(
        128 * _j, 128 * _j + 128
    )
_PERM[1280:1344] = np.arange(640, 704)
_PERM[1344:1408] = INTER + np.arange(640, 704)


def _to_np_dt(a):
    """Cast fp32 array to the host dtype for MM_DT."""
    if MM_DT == mybir.dt.bfloat16:
        b = np.asarray(a, dtype=np.float32).copy()
        v = b.view(np.uint32)
        v += 0x8000  # round-to-nearest-even-ish
        v &= 0xFFFF0000
        return b
    return np.asarray(a, dtype=NP_DT)


def _make_chunks(counts, starts, tmax):
    chunks = []  # (n, expert, tok_start)
    for e in range(N_EXPERTS):
        n = int(counts[e])
        a = int(starts[e])
        if n <= 0:
            continue
        nparts = -(-n // tmax)
        base, rem = divmod(n, nparts)
        off = 0
        for p in range(nparts):
            ln = base + (1 if p < rem else 0)
            if ln > 0:
                chunks.append((ln, e, a + off))
                off += ln
    return chunks


def _pe_time(caps):
    """Predicted PE critical time for a segment-cap structure (fp16)."""
    col = 0.427e-9  # per moving column at 2.4GHz
    ld = 88e-9  # LDWEIGHTS (measured on HW, FWL fp16)
    t = 0.0
    for C in caps:
        for t0 in range(0, C, TT):
            tt = min(TT, C - t0)
            t += 88 * max(tt * col, ld)  # GEMM1: 8k x 11m stationary loads
            t += -(-tt // 128) * 12 * max(512 * col, ld)  # GEMM2 chunks
    return t


def _dma_time(S, cap_total):
    w_seg = (HIDDEN * 2 * INTER + 768 * HIDDEN) * ESZ  # w13 + padded w2
    out_b = 2 if OUT_F16 else 4
    return (S * w_seg + cap_total * HIDDEN * (ESZ + out_b)) / 390e9


def _plan(counts):
    """Balance (expert, token-chunk) pieces across NCORES cores.

    Chunks are sorted by size and dealt in bands of 8 (one per core): slot s
    capacity = the largest chunk in band s, which minimizes total capacity
    for a given chunk multiset. The split threshold trades segment count
    (weight DMA traffic) against padding (PE + activation traffic).
    """
    starts = np.zeros(N_EXPERTS, dtype=np.int64)
    np.cumsum(counts[:-1], out=starts[1:])

    best = None
    for tmax in (4096, 2048, 1536, 1024, *range(256, 1025, 16)):
        chunks = _make_chunks(counts, starts, max(1, tmax))
        if not chunks:
            chunks = [(0, None, 0)]
        chunks.sort(key=lambda c: -c[0])
        S = -(-len(chunks) // NCORES)
        caps = []
        for s in range(S):
            band = chunks[NCORES * s : NCORES * (s + 1)]
            caps.append(max(8, ((band[0][0] + 7) // 8) * 8))
        cap_total = sum(caps)
        dma_t = _dma_time(S, cap_total)
        pe_t = _pe_time(caps)
        score = max(dma_t, pe_t) + 0.2 * min(dma_t, pe_t)
        if best is None or score < best[0]:
            best = (score, chunks, S, caps)

    _, chunks, S, caps = best
    offs = np.concatenate([[0], np.cumsum(caps)[:-1]]).astype(np.int64)
    cap_total = int(sum(caps))

    assign = [[] for _ in range(NCORES)]
    for s in range(S):
        band = chunks[NCORES * s : NCORES * (s + 1)]
        for c in range(NCORES):
            if c < len(band):
                n, e, a = band[c]
                assign[c].append((e, a, n))
            else:
                assign[c].append((None, 0, 0))
    return assign, caps, offs, cap_total


def _tiles_of(caps):
    """Token tiles as (segment, t0, tt) in execution order."""
    out = []
    for s, C in enumerate(caps):
        for t0 in range(0, C, TT):
            out.append((s, t0, min(TT, C - t0)))
    return out


def _build(S, caps, cap_total):
    """Build the SPMD Bass program for one core's segment structure."""
    nc = bacc.Bacc(
        "TRN2",
        target_bir_lowering=False,
        debug=False,
        enable_asserts=False,
        num_devices=NCORES,
    )

    tiles = _tiles_of(caps)
    NT = len(tiles)
    offs = np.concatenate([[0], np.cumsum(caps)[:-1]]).astype(np.int64)
    out_dt = f16 if OUT_F16 else f32

    xt_d = nc.declare_dram_parameter("xt", [NT, 128, KC * TT], MM_DT, isOutput=False)
    w13_d = nc.declare_dram_parameter(
        "w13", [S, 4, 128, 2 * 2 * INTER], MM_DT, isOutput=False
    )
    w2_d = nc.declare_dram_parameter(
        "w2", [S, 128, JC * HIDDEN], MM_DT, isOutput=False
    )
    out_d = nc.declare_dram_parameter("out", [cap_total, HIDDEN], out_dt, isOutput=True)

    # SBUF pool sizing: slots scale with ESZ; keep total under ~23MB.
    big = ESZ == 4
    w13_bufs = 6 if big else 12
    w2_bufs = 2 if big else 3
    xt_bufs = 3 if big else 4
    hp_bufs = 12 if big else 18

    with tile.TileContext(nc) as tc:
        with (
            tc.tile_pool(name="w13p", bufs=w13_bufs) as w13p,
            tc.tile_pool(name="w2p", bufs=w2_bufs) as w2p,
            tc.tile_pool(name="xtp", bufs=xt_bufs) as xtp,
            tc.tile_pool(name="hp", bufs=hp_bufs) as hp,
            tc.tile_pool(name="sgp", bufs=6) as sgp,
            tc.tile_pool(name="outp", bufs=4) as outp,
            tc.tile_pool(name="ps1", bufs=4, space="PSUM") as ps1,
            tc.tile_pool(name="ps2", bufs=2, space="PSUM") as ps2,
        ):
            # No HAM warmup: the PE clock ramps (1.2GHz -> 2.4GHz after
            # ~3.4us sustained activity) during the opening wire-gated
            # weight staircase, where the PE is DMA-bound anyway — cold
            # matmuls there cost no end-time, and skipping warmup lets the
            # real stream start the moment pair0+xt land.

            # Deferred GEMM2 work queue: closures emitted between the next
            # segment's GEMM1 m-groups (fills LDWEIGHTS-bound PE stalls).
            g2q = []

            def pop_g2():
                if g2q:
                    g2q.pop(0)()

            # xt tiles, issued one segment ahead of use so the wire order is
            # [seg s weights][seg s+1 xt][seg s+1 weights]... and GEMM1 of
            # seg s+1 never waits on its activations.
            xt_tiles = {}
            seg_first_tile = {}
            for i, (ts, _, _) in enumerate(tiles):
                seg_first_tile.setdefault(ts, i)

            def issue_xt(i):
                if i in xt_tiles or i >= NT:
                    return
                _, _, tt_i = tiles[i]
                xtt = xtp.tile([128, KC * tt_i], MM_DT, tag="xtt",
                               name=f"xtt{i}", padded_shape=[128, KC * TT])
                nc.sync.dma_start(out=xtt[:], in_=xt_d[i, :, 0 : KC * tt_i])
                xt_tiles[i] = xtt

            tix = 0
            for s in range(S):
                C = caps[s]
                off = int(offs[s])

                # Weight DMAs in first-use (k) order; for the first segment
                # the first token tile's xt goes right after pair 0 so the
                # opening k-outer group consumes (pair0, xt, pair1, ...) as
                # they land on the wire.
                w13_t = []

                def load_pair(kp, s=s, w13_t=w13_t):
                    w13t = w13p.tile([128, 2 * 2 * INTER], MM_DT, tag="w13t",
                                     name=f"w13t{s}_{kp}")
                    nc.sync.dma_start(out=w13t[:], in_=w13_d[s, kp])
                    w13_t.append(w13t)

                if s == 0:
                    # Startup race: split pair0 and xt0 into k-halves so the
                    # opening k-loop's first matmuls start after ~0.7MB on
                    # the wire instead of ~1.4MB. Disjoint-slice DMAs into
                    # one tile: readers of the k=0 half wait only on DMA 0
                    # (view-overlap hazard tracking is slice-granular).
                    w13t0 = w13p.tile([128, 2 * 2 * INTER], MM_DT,
                                      tag="w13t", name="w13t0_0")
                    nc.sync.dma_start(out=w13t0[:, 0 : 2 * INTER],
                                      in_=w13_d[0, 0, :, 0 : 2 * INTER])
                    w13_t.append(w13t0)
                    tt0 = tiles[0][2]
                    xh = (KC // 2) * tt0
                    xtt0 = xtp.tile([128, KC * tt0], MM_DT, tag="xtt",
                                    name="xtt0", padded_shape=[128, KC * TT])
                    nc.sync.dma_start(out=xtt0[:, 0:xh], in_=xt_d[0, :, 0:xh])
                    xt_tiles[0] = xtt0
                    nc.sync.dma_start(
                        out=w13t0[:, 2 * INTER : 4 * INTER],
                        in_=w13_d[0, 0, :, 2 * INTER : 4 * INTER],
                    )
                    load_pair(1)
                    nc.sync.dma_start(out=xtt0[:, xh : KC * tt0],
                                      in_=xt_d[0, :, xh : KC * tt0])
                    load_pair(2)
                    load_pair(3)
                else:
                    for kp in range(4):
                        load_pair(kp)
                w2t = w2p.tile([128, JC * HIDDEN], MM_DT, tag="w2t", name=f"w2t{s}")
                nc.sync.dma_start(out=w2t[:], in_=w2_d[s])
                # Remaining tiles of this segment, then prefetch the next
                # segment's first tile.
                for i in range(seg_first_tile[s], NT):
                    if tiles[i][0] != s:
                        break
                    issue_xt(i)
                if s + 1 < S:
                    issue_xt(seg_first_tile[s + 1])

                def w13_ap(k, m, w13_t=w13_t):
                    base = (k % 2) * 2 * INTER + 128 * m
                    return w13_t[k // 2][:, base : base + 128]

                def w2_ap(j, nn, w2t=w2t):
                    # Full 128 rows even for the j=5 tail: rows 704:768 are
                    # host-packed zeros and h's pad rows are memset to zero,
                    # so the matmul result is unchanged but the stationary
                    # is 128-wide (FWL eligible).
                    base = j * HIDDEN + 512 * nn
                    return w2t[:, base : base + 512]

                for t0 in range(0, C, TT):
                    tt = min(TT, C - t0)
                    xt_tile = xt_tiles[tix]

                    def xt_ap(k, xt_tile=xt_tile, tt=tt):
                        return xt_tile[:, k * tt : (k + 1) * tt]

                    h_t = []
                    for j in range(JC):
                        jw = min(128, INTER - 128 * j)
                        if jw < 128:
                            # Allocate the tail h chunk full-height and zero
                            # the pad rows: GEMM2 can then use a 128-column
                            # stationary (FWL eligible, ~50ns faster per
                            # matmul); the pad rows hit w2's zero pad rows.
                            ht = hp.tile([128, tt], MM_DT, tag="ht",
                                         name=f"ht{tix}_{j}",
                                         padded_shape=[128, TT])
                            nc.vector.memset(ht[jw:128, :], 0.0)
                        else:
                            ht = hp.tile([jw, tt], MM_DT, tag="ht",
                                         name=f"ht{tix}_{j}",
                                         padded_shape=[128, TT])
                        h_t.append(ht)

                    # GEMM1: k-outer within m-groups. The first token tile
                    # races the initial weight DMAs: one wide group of 8
                    # m-chunks (borrowing the two ps2 tiles as 4 psum-bank
                    # halves) gives the PE ~2.3us of matmul per arriving
                    # w13 k-pair (~1.8us wire each), hiding the staircase.
                    if tix == 0:
                        # (8,10)+(10,11) split: the trailing groups reuse
                        # psum tiles freed pair-by-pair by the big group's
                        # silu/mul, starting ~0.7us earlier than one
                        # 3-tile group could.
                        m_groups = [(0, 8), (8, 10), (10, MC)]
                    else:
                        m_groups = M_GROUPS
                    for m_lo, m_hi in m_groups:
                        # Pop a deferred GEMM2 chunk first: it is ready to
                        # run and keeps the PE fed while this group's first
                        # matmul may still be waiting on weights.
                        pop_g2()
                        pg_t = {}  # m -> (tile, col offset)
                        for m in range(m_lo, m_hi):
                            if m - m_lo < 4 or m_hi - m_lo <= 4:
                                pt = ps1.tile([128, tt], f32, tag="pg",
                                              name=f"pg{m}",
                                              padded_shape=[128, TT])
                                pg_t[m] = (pt, 0)
                            elif (m - m_lo) % 2 == 0:
                                bt = ps2.tile([128, 512 + tt], f32, tag="po",
                                              name=f"pgb{m}",
                                              padded_shape=[128, HIDDEN])
                                pg_t[m] = (bt, 0)
                            else:
                                pg_t[m] = (pg_t[m - 1][0], 512)

                        def pg_ap(m, r0=0, r1=128):
                            pt, c0 = pg_t[m]
                            return pt[r0:r1, c0 : c0 + tt]

                        for k in range(KC):
                            for m in range(m_lo, m_hi):
                                nc.tensor.matmul(
                                    pg_ap(m),
                                    w13_ap(k, m),
                                    xt_ap(k),
                                    start=(k == 0),
                                    stop=(k == KC - 1),
                                )
                        for m in range(m_lo, m_hi):
                            if m == MC - 1:
                                # tail chunk: [gate 64 | up 64] on partitions
                                sg = sgp.tile([64, tt], f32, tag="sg",
                                              name=f"sg{m}",
                                              padded_shape=[128, TT])
                                nc.scalar.activation(
                                    sg[:], pg_ap(m, 0, 64),
                                    mybir.ActivationFunctionType.Silu,
                                )
                                nc.vector.tensor_mul(
                                    h_t[JC - 1][0:64, :], sg[:],
                                    pg_ap(m, 64, 128),
                                )
                            elif m % 2 == 1:
                                # full pair: chunk m-1 = gate, chunk m = up
                                sg = sgp.tile([128, tt], f32, tag="sg",
                                              name=f"sg{m}",
                                              padded_shape=[128, TT])
                                nc.scalar.activation(
                                    sg[:], pg_ap(m - 1),
                                    mybir.ActivationFunctionType.Silu,
                                )
                                nc.vector.tensor_mul(
                                    h_t[m // 2][:], sg[:], pg_ap(m)
                                )

                    # GEMM2: h stationary, w2 moving; token-major output.
                    # Deferred: emitted between the NEXT segment's m-groups.
                    # The last segment (end of the program) runs nn-split:
                    # two [tw, 512] psum halves so the cast/store of half 0
                    # overlaps the matmuls of half 1, shortening the final
                    # dependency chain GEMM2 -> cast -> store.
                    def g2_chunk(tc0, tt=tt, h_t=h_t, w2_ap=w2_ap, off=off,
                                 t0=t0, s=s):
                        tw = min(128, tt - tc0)
                        row0 = off + t0 + tc0
                        store_eng = nc.sync if s == S - 1 else nc.gpsimd
                        if s == S - 1:
                            for nn in range(HIDDEN // 512):
                                po = ps2.tile([tw, 512], f32, tag="po",
                                              name="po", padded_shape=[128, HIDDEN])
                                for j in range(JC):
                                    nc.tensor.matmul(
                                        po[:],
                                        h_t[j][:, tc0 : tc0 + tw],
                                        w2_ap(j, nn),
                                        start=(j == 0),
                                        stop=(j == JC - 1),
                                    )
                                ob = outp.tile([tw, 512], out_dt, tag="ob",
                                               name="ob", padded_shape=[128, HIDDEN])
                                nc.vector.tensor_copy(ob[:], po[:])
                                store_eng.dma_start(
                                    out=out_d[row0 : row0 + tw,
                                              512 * nn : 512 * (nn + 1)],
                                    in_=ob[:],
                                )
                            return
                        po = ps2.tile([tw, HIDDEN], f32, tag="po", name="po",
                                      padded_shape=[128, HIDDEN])
                        for j in range(JC):
                            for nn in range(HIDDEN // 512):
                                nc.tensor.matmul(
                                    po[:, 512 * nn : 512 * (nn + 1)],
                                    h_t[j][:, tc0 : tc0 + tw],
                                    w2_ap(j, nn),
                                    start=(j == 0),
                                    stop=(j == JC - 1),
                                )
                        ob = outp.tile([tw, HIDDEN], out_dt, tag="ob", name="ob",
                                       padded_shape=[128, HIDDEN])
                        nc.vector.tensor_copy(ob[:], po[:])
                        # Stores ride the gpsimd (SWDGE) queue so they never
                        # block later loads on the sync sequencer; the last
                        # segment has no loads after it, so its stores take
                        # the faster HWDGE path.
                        store_eng.dma_start(
                            out=out_d[row0 : row0 + tw, :],
                            in_=ob[:],
                        )

                    for tc0 in range(0, tt, 128):
                        g2q.append(lambda tc0=tc0: g2_chunk(tc0))
                    tix += 1

            while g2q:
                pop_g2()

    nc.compile()
    return nc


_BUILD_CACHE = {}


def _get_program(S, caps, cap_total):
    key = (S, tuple(caps), str(MM_DT), OUT_F16)
    if key not in _BUILD_CACHE:
        _BUILD_CACHE[key] = _build(S, caps, cap_total)
    return _BUILD_CACHE[key]


def _pack_inputs(x, assign, caps, offs, cap_total, w13_perm, w2):
    """Build per-core input dicts matching the device layouts."""
    tiles = _tiles_of(caps)
    NT = len(tiles)
    S = len(caps)
    in_maps = []
    for c in range(NCORES):
        xt_c = np.zeros((HIDDEN, cap_total), dtype=NP_DT)
        w13_c = np.zeros((S, 4, 128, 2 * 2 * INTER), dtype=NP_DT)
        w2_c = np.zeros((S, 128, JC * HIDDEN), dtype=NP_DT)
        for s, (e, a, n) in enumerate(assign[c]):
            if e is None or n <= 0:
                continue
            o = int(offs[s])
            xt_c[:, o : o + n] = _to_np_dt(x[a : a + n, :]).T
            # w13: [1024, 1408] -> [4, 2, 128, 1408] -> [4, 128, 2*1408]
            w13_c[s] = (
                w13_perm["w13"][e]
                .reshape(4, 2, 128, 2 * INTER)
                .transpose(0, 2, 1, 3)
                .reshape(4, 128, 2 * 2 * INTER)
            )
            # w2: pad [704,1024] -> [768,1024] -> [6,128,1024] -> [128, 6*1024]
            w2_c[s] = w13_perm["w2"][e]
        # xt: per token tile [1024, tt] -> [8, 128, tt] -> [128, 8*tt]
        xt_pack = np.zeros((NT, 128, KC * TT), dtype=NP_DT)
        for tix, (s, t0, tt) in enumerate(tiles):
            o = int(offs[s])
            blk = xt_c[:, o + t0 : o + t0 + tt]  # [1024, tt]
            xt_pack[tix, :, 0 : KC * tt] = (
                blk.reshape(KC, 128, tt).transpose(1, 0, 2).reshape(128, KC * tt)
            )
        in_maps.append({"xt": xt_pack, "w13": w13_c, "w2": w2_c})
    return in_maps


def _prep_weights(w1w3, w2):
    """Permute/pack weights once (shared across cores)."""
    w13_perm = _to_np_dt(w1w3[:, :, _PERM])  # [E, HIDDEN, 2*INTER]
    w2p_all = np.zeros((N_EXPERTS, 768, HIDDEN), dtype=NP_DT)
    w2p_all[:, :INTER] = _to_np_dt(w2)
    w2_pack = (
        w2p_all.reshape(N_EXPERTS, JC, 128, HIDDEN)
        .transpose(0, 2, 1, 3)
        .reshape(N_EXPERTS, 128, JC * HIDDEN)
    )
    return {"w13": w13_perm, "w2": w2_pack}


def _run(x, tokens_per_expert, w1w3, w2, trace=False):
    x = np.ascontiguousarray(np.asarray(x, dtype=np.float32))
    counts = np.asarray(tokens_per_expert, dtype=np.int64).copy()
    w1w3 = np.asarray(w1w3, dtype=np.float32)
    w2 = np.asarray(w2, dtype=np.float32)

    T = x.shape[0]
    # Clip group sizes like ragged_dot: groups are consecutive; anything
    # beyond T is out of range.
    counts = np.maximum(counts, 0)
    cum = np.cumsum(counts)
    over = cum > T
    if over.any():
        first = int(np.argmax(over))
        prev = int(cum[first - 1]) if first > 0 else 0
        counts[first] = T - prev
        counts[first + 1 :] = 0

    assign, caps, offs, cap_total = _plan(counts)
    S = len(caps)
    nc = _get_program(S, caps, cap_total)

    packed_w = _prep_weights(w1w3, w2)
    in_maps = _pack_inputs(x, assign, caps, offs, cap_total, packed_w, w2)

    extra = {}
    if trace:
        import os

        os.makedirs("/tmp/moe_prof", exist_ok=True)
        for f in os.listdir("/tmp/moe_prof"):
            os.unlink(os.path.join("/tmp/moe_prof", f))
        extra["tmpdir"] = "/tmp/moe_prof"
    res = run_bass_kernel_spmd(nc, in_maps, list(range(NCORES)), trace=trace, **extra)

    out_full = np.zeros((T, HIDDEN), dtype=np.float32)
    for c in range(NCORES):
        oc = res.results[c]["out"]
        for s, (e, a, n) in enumerate(assign[c]):
            if e is None or n <= 0:
                continue
            o = int(offs[s])
            out_full[a : a + n, :] = oc[o : o + n, :].astype(np.float32)
    return out_full, res


def kernel(x, tokens_per_expert, w1w3, w2, decoding=False, **_ignored):
    out, _ = _run(x, tokens_per_expert, w1w3, w2, trace=False)
    return out
